# revision 1
# baseline (speedup 1.0000x reference)
"""Trainium2 Bass kernel for nn_Block_41077067219413.

Reference computation (B=2048, D=dim_in=4096, J=dim_out=4096):
    xf = x.astype(f32)                 # (B, D) in {0,1}
    mf = masks.astype(f32)             # (D, J) in {0,1}
    sums = xf @ mf + (1-xf) @ (1-mf)   # XNOR popcount over D
    out  = sums > thresholds[None, :]  # (B, J) bool

Identity used: with x' = 2x-1 in {-1,+1} and m in {0,1},
    A[b,j] = sum_k x'[b,k] * m[k,j]
    sums   = A + D - rowsum_x[b]
    out    = A - th[j] > rowsum_x[b] - D

One fp8 GEMM per core (batch-sharded 8 ways).  masks bytes {0,1} are DMA'd
raw and bitcast to fp8e4, where 0x01 is the subnormal eps=2^-9 -- the GEMM
computes eps*A exactly (integers scaled by eps are exact in fp32).
Thresholds are folded into the GEMM as 4 extra contraction rows carrying
base-8 digits of th with eps-scaled weights, so PSUM = eps*(A - th).
Epilogue: single per-partition-scalar is_gt vs eps*(rowsum_x - D) -> uint8.
"""

import numpy as np

B, D, J = 2048, 4096, 4096
NCORES = 8
BL = B // NCORES          # 256 rows per core
P = 128
KT = D // P               # 32 k-tiles
NB = BL // P              # 2 b-tiles per core
JN = 512                  # matmul free-dim tile (one PSUM bank)
JT = J // JN              # 8 j-tiles
THC = 1024                # threshold digit-build chunk width

_cache = {}


def _build():
    import concourse.bacc as bacc
    import concourse.mybir as mybir
    import concourse.tile as tile

    dt = mybir.dt
    f8 = dt.float8e4
    f32 = dt.float32
    AF = mybir.ActivationFunctionType
    ALU = mybir.AluOpType
    DR = mybir.MatmulPerfMode.DoubleRow

    nc = bacc.Bacc("TRN2", target_bir_lowering=False, debug=False,
                   num_devices=NCORES)

    x_d = nc.dram_tensor("x", [BL, D], dt.int32, kind="ExternalInput")
    m_d = nc.dram_tensor("masks", [D, J], dt.uint8, kind="ExternalInput")
    th_d = nc.dram_tensor("th", [1, J], dt.int32, kind="ExternalInput")
    cst8_d = nc.dram_tensor("cst8", [4, P], f8, kind="ExternalInput")
    ident_d = nc.dram_tensor("ident", [P, P], f8, kind="ExternalInput")
    csts_d = nc.dram_tensor("csts", [4, 2], dt.int32, kind="ExternalInput")
    o_d = nc.dram_tensor("out", [BL, J], dt.uint8, kind="ExternalOutput")

    with tile.TileContext(nc) as tc:
        with (
            tc.tile_pool(name="const", bufs=1) as constp,
            tc.tile_pool(name="mask", bufs=1) as maskp,
            tc.tile_pool(name="xt", bufs=1) as xtp,
            tc.tile_pool(name="ob", bufs=2) as obufp,
            tc.tile_pool(name="xio", bufs=4) as xiop,
            tc.tile_pool(name="thp", bufs=1) as thp,
            tc.tile_pool(name="xpm", bufs=1) as xpmp,
        ):
            # ---- x loads lead the sync (HWDGE) queue: critical path to PE
            # (half-tiles so the convert/transpose chain starts early)
            HD = D // 2
            xio = {}
            for b in range(NB):
                for h in range(2):
                    xi = xiop.tile([P, HD], dt.int32, tag="xi",
                                   name=f"xi{b}_{h}")
                    nc.gpsimd.dma_start(
                        xi[:], x_d[b * P:(b + 1) * P, h * HD:(h + 1) * HD])
                    xio[(b, h)] = xi

            # ---- const tables (small sync DMAs)
            wstar = constp.tile([4, P], f8)
            nc.scalar.dma_start(wstar[:], cst8_d[:])
            identity8 = constp.tile([P, P], f8)
            nc.scalar.dma_start(identity8[:], ident_d[:])
            shiftands = constp.tile([4, 2], dt.int32)
            nc.scalar.dma_start(shiftands[:], csts_d[:])

            neg1 = constp.tile([P, 1], f32)
            nc.vector.memset(neg1[:], -1.0)
            actwarm = constp.tile([P, 1], f32)
            nc.scalar.activation(actwarm[:], neg1[:], AF.Identity,
                                 bias=neg1[:], scale=1.0)
            rxe = constp.tile([P, NB], f32)
            dig8 = constp.tile([4, J], f8)

            # ---- masks: raw uint8 DMA, k-pair layout (bitcast fp8 at use)
            xgate = constp.tile([1, 4], dt.int32)
            nc.gpsimd.tensor_copy(xgate[:], xio[(1, 1)][0:1, 0:4])
            JH = J // 2
            mask_tiles = {}
            for jh in range(2):
                for kp in range(KT // 2):
                    mt = maskp.tile([P, 2, JH], dt.uint8,
                                    name=f"mk{jh}_{kp}", tag=f"mk{jh}_{kp}")
                    src = m_d[kp * 2 * P:(kp + 1) * 2 * P,
                              jh * JH:(jh + 1) * JH].rearrange(
                        "(ko ki) j -> ki ko j", ki=P)
                    nc.gpsimd.dma_start(mt[:], src)
                    mask_tiles[(jh, kp)] = mt

            # ---- thresholds -> base-8 digit rows [4, J] fp8 (chunked temps)
            if True:
                for c0 in range(0, J, THC):
                    th4 = thp.tile([4, THC], dt.int32, tag="th4",
                                   name=f"th4_{c0}")
                    for i in range(4):
                        nc.scalar.dma_start(th4[i:i + 1, :],
                                            th_d[:, c0:c0 + THC])
                    dig_i = thp.tile([4, THC], dt.int32, tag="dig_i",
                                     name=f"dig_i_{c0}")
                    nc.vector.tensor_scalar(
                        dig_i[:], th4[:], shiftands[:, 0:1],
                        shiftands[:, 1:2],
                        op0=ALU.arith_shift_right, op1=ALU.bitwise_and)
                    nc.vector.tensor_copy(dig8[:, c0:c0 + THC], dig_i[:])

            # ---- x: int32 -> fp8 {-1,+1} + rowsum (convert now,
            # transposes emitted inside the main section where they share
            # PSUM slots with the b1 accumulator tags)
            xT = xtp.tile([P, KT, NB * P], f8)
            xpms = {}
            for b in range(NB):
                rxas = []
                for h in range(2):
                    xpm = xpmp.tile([P, HD], f8, tag=f"xpm{b}_{h}",
                                    name=f"xpm{b}_{h}")
                    rxa = xpmp.tile([P, 1], f32, tag=f"rxa{b}_{h}",
                                    name=f"rxa{b}_{h}")
                    nc.scalar.activation(
                        xpm[:], xio[(b, h)][:], AF.Identity,
                        bias=neg1[:], scale=2.0, accum_out=rxa[:])
                    xpms[(b, h)] = xpm
                    rxas.append(rxa)
                nc.vector.tensor_tensor(
                    rxas[0][:], rxas[0][:], rxas[1][:], op=ALU.add)
                nc.vector.tensor_scalar(
                    rxe[:, b:b + 1], rxas[0][:], 1.0 / 1024.0, -4.0,
                    op0=ALU.mult, op1=ALU.add)

            # ---- main GEMM + fused threshold + epilogue
            obs = [obufp.tile([P, J], dt.uint8, tag=f"ob{b}", name=f"ob{b}")
                   for b in range(NB)]
            with tc.tile_pool(name="psacc", bufs=1, space="PSUM") as psacc:

                def transposes(b):
                    # pst tiles share the b1-accumulator slots (acc1_*)
                    for h in range(2):
                        for pp in range(KT // 4):
                            pst = psacc.tile(
                                [P, 2, P, 2], f8,
                                tag=f"acc1_{pp % 4}",
                                name=f"pst{b}_{h}_{pp}")
                            for q in range(2):
                                k = 2 * pp + q
                                nc.tensor.transpose(
                                    pst[:, q, :, 0],
                                    xpms[(b, h)][:, k * P:(k + 1) * P],
                                    identity8[:])
                            kk = h * (KT // 2) + 2 * pp
                            nc.vector.tensor_copy(
                                xT[:, kk:kk + 2, b * P:(b + 1) * P],
                                pst[:, :, :, 0])

                KH = KT // 4     # 8: first half of kp range (kp-major)
                KW = 4           # warm-up kp rows emitted between T phases
                for jh in range(2):
                    ps = {}
                    for j4 in range(4):
                        ps[(0, j4)] = psacc.tile(
                            [P, JN], f32, name=f"acc{jh}_0_{j4}",
                            tag=f"acc0_{j4}")
                    if jh == 0:
                        transposes(0)
                        for kp in range(KW):
                            mt = mask_tiles[(jh, kp)]
                            w = xT[:, 2 * kp:2 * kp + 2, 0:P]
                            for j4 in range(4):
                                nc.tensor.matmul(
                                    ps[(0, j4)][:], w,
                                    mt[:, :,
                                       j4 * JN:(j4 + 1) * JN].bitcast(f8),
                                    start=(kp == 0), stop=False,
                                    perf_mode=DR)
                        transposes(1)
                    for j4 in range(4):
                        ps[(1, j4)] = psacc.tile(
                            [P, JN], f32, name=f"acc{jh}_1_{j4}",
                            tag=f"acc1_{j4}")
                    if jh == 0:
                        for kp in range(KW):
                            mt = mask_tiles[(jh, kp)]
                            w = xT[:, 2 * kp:2 * kp + 2, P:2 * P]
                            for j4 in range(4):
                                nc.tensor.matmul(
                                    ps[(1, j4)][:], w,
                                    mt[:, :,
                                       j4 * JN:(j4 + 1) * JN].bitcast(f8),
                                    start=(kp == 0), stop=False,
                                    perf_mode=DR)
                    for kp in range(KW if jh == 0 else 0, KH):
                        mt = mask_tiles[(jh, kp)]
                        for b in range(NB):
                            w = xT[:, 2 * kp:2 * kp + 2, b * P:(b + 1) * P]
                            for j4 in range(4):
                                nc.tensor.matmul(
                                    ps[(b, j4)][:], w,
                                    mt[:, :,
                                       j4 * JN:(j4 + 1) * JN].bitcast(f8),
                                    start=(kp == 0), stop=False,
                                    perf_mode=DR)
                        if kp == 6:
                            # fold thresholds: psum -= eps*th
                            for b in range(NB):
                                for j4 in range(4):
                                    jj = jh * (J // 2) + j4 * JN
                                    nc.tensor.matmul(
                                        ps[(b, j4)][:], wstar[:],
                                        dig8[:, jj:jj + JN],
                                        start=False, stop=False,
                                        skip_group_check=True)
                    # second k-half group-major: groups retire staggered so
                    # the is_gt epilogue overlaps remaining matmuls
                    for b in range(NB):
                        for j4 in range(4):
                            jj = jh * (J // 2) + j4 * JN
                            w = None
                            for kp in range(KH, KT // 2):
                                nc.tensor.matmul(
                                    ps[(b, j4)][:],
                                    xT[:, 2 * kp:2 * kp + 2,
                                       b * P:(b + 1) * P],
                                    mask_tiles[(jh, kp)][
                                        :, :,
                                        j4 * JN:(j4 + 1) * JN].bitcast(f8),
                                    start=False, stop=(kp == KT // 2 - 1),
                                    perf_mode=DR)
                            nc.vector.tensor_scalar(
                                obs[b][:, jj:jj + JN], ps[(b, j4)][:],
                                rxe[:, b:b + 1], None, op0=ALU.is_gt)
                            nc.sync.dma_start(
                                o_d[b * P:(b + 1) * P, jj:jj + JN],
                                obs[b][:, jj:jj + JN])

    nc.compile()
    return nc


def _get_nc():
    if "nc" not in _cache:
        _cache["nc"] = _build()
    return _cache["nc"]


def _cst8():
    import ml_dtypes
    # eps-scaled digit weights: -eps*8^i per digit row (row 3 holds 8*d3,
    # so its weight is -eps*512/8 = -2^-3)
    w = np.array([-2.0 ** -9, -2.0 ** -6, -2.0 ** -3, -2.0 ** -3],
                 dtype=np.float32)
    return np.broadcast_to(w[:, None], (4, P)).astype(ml_dtypes.float8_e4m3)


def _ident():
    import ml_dtypes
    return np.eye(P, dtype=np.float32).astype(ml_dtypes.float8_e4m3)


def _csts():
    return np.array([[0, 7], [3, 7], [6, 7], [6, 56]], dtype=np.int32)


def run(x, masks, thresholds, trace=False):
    """Run the SPMD kernel on 8 cores. Returns (out_bool, BassKernelResults)."""
    from concourse.bass_utils import run_bass_kernel_spmd

    nc = _get_nc()
    m_u8 = np.ascontiguousarray(masks.view(np.uint8))
    th = np.ascontiguousarray(thresholds.reshape(1, J).astype(np.int32))
    in_maps = []
    for c in range(NCORES):
        in_maps.append({
            "x": np.ascontiguousarray(x[c * BL:(c + 1) * BL, :]),
            "masks": m_u8,
            "th": th,
            "cst8": _cst8(),
            "ident": _ident(),
            "csts": _csts(),
        })
    res = run_bass_kernel_spmd(nc, in_maps, core_ids=list(range(NCORES)),
                               trace=trace)
    out = np.concatenate([r["out"] for r in res.results], axis=0)
    return out.view(np.bool_), res


def kernel(x, masks, thresholds):
    x = np.asarray(x)
    masks = np.asarray(masks)
    thresholds = np.asarray(thresholds)
    out, _ = run(x, masks, thresholds, trace=False)
    return out



# revision 8
# speedup vs baseline: 1.2292x; 1.2292x over previous
"""Trainium2 Bass kernel for nn_Block_41077067219413.

Reference computation (B=2048, D=dim_in=4096, J=dim_out=4096):
    xf = x.astype(f32)                 # (B, D) in {0,1}
    mf = masks.astype(f32)             # (D, J) in {0,1}
    sums = xf @ mf + (1-xf) @ (1-mf)   # XNOR popcount over D
    out  = sums > thresholds[None, :]  # (B, J) bool

Identity: with x' = 2x-1 in {-1,+1}, m in {0,1}, A = x' @ m:
    sums = A + D - rowsum_x   (colsum terms cancel)
    out  = A > th[j] + rowsum_x[b] - D

Sharding: 4 batch groups x 2 j-halves across 8 cores.  Per core one fp8
DoubleRow GEMM [512 x 4096] @ [4096 x 2048] -- 256 matmuls of
[K=256]x[N=512], the PE-array floor (~55us at 157 TF/s fp8-DR).
Everything else is kept off the PE:
  - x is host-marshalled to the exact stationary tile layout (fp8 +-1,
    transposed, DR k-pairing) -- no on-device transposes/converts.
  - masks DMA'd raw as uint8 in k-pair tile layout, bitcast to fp8
    (byte 0x01 == eps = 2^-9 subnormal); psum accumulates eps*A exactly.
  - thresholds ship as an eps-scaled f32 broadcast tile; epilogue is
    tmp = psum - eps*th (DVE, releases the psum bank) then
    out = tmp > eps*(rowsum-D) (scalar/gpsimd), all integer-exact.
  - rowsum_x comes from a row-major fp8 x copy via accum-reductions
    spread over the scalar/vector/gpsimd engines mid-flight.
PSUM: two waves of 8 banks (2 b-tiles x 4 j-tiles each), kp 0..11
kp-major (tiles consumed in DMA arrival order), kp 12..15 group-major
so groups retire staggered.  Dummy warm-up matmuls ramp the PE p-state
while the first tiles land.
"""

import numpy as np

B, D, J = 2048, 4096, 4096
NCORES = 8
GB = 4                    # batch groups
GJ = 2                    # j halves
ML = B // GB              # 512 rows per core
JL = J // GJ              # 2048 cols per core
P = 128
NB = ML // P              # 4 b-tiles per core
KP = D // 256             # 16 k-pair tiles
JN = 512                  # one PSUM bank
JT = JL // JN             # 4 j-tiles
KRET = 4                  # retirement kps (12..15)
WARM = 14                 # PE p-state warm-up matmuls

_cache = {}


def _build():
    import concourse.bacc as bacc
    import concourse.mybir as mybir
    import concourse.tile as tile

    dt = mybir.dt
    f8 = dt.float8e4
    f32 = dt.float32
    AF = mybir.ActivationFunctionType
    ALU = mybir.AluOpType
    DR = mybir.MatmulPerfMode.DoubleRow

    nc = bacc.Bacc("TRN2", target_bir_lowering=False, debug=False,
                   num_devices=NCORES)

    xT_d = nc.dram_tensor("xT", [NB, P, KP, 2, P], f8, kind="ExternalInput")
    xrm_d = nc.dram_tensor("xrm", [NB, P, D], f8, kind="ExternalInput")
    m_d = nc.dram_tensor("masks", [KP, P, 2, JL], dt.uint8,
                         kind="ExternalInput")
    thb_d = nc.dram_tensor("thb", [P, JL], f32, kind="ExternalInput")
    o_d = nc.dram_tensor("out", [NB, P, JL], dt.uint8, kind="ExternalOutput")

    with tile.TileContext(nc) as tc:
        with (
            tc.tile_pool(name="const", bufs=1) as constp,
            tc.tile_pool(name="mask", bufs=1) as maskp,
            tc.tile_pool(name="xt", bufs=1) as xtp,
            tc.tile_pool(name="xrm", bufs=1) as xrmp,
            tc.tile_pool(name="acts", bufs=1) as actp,
            tc.tile_pool(name="tmp", bufs=1) as tmpp,
            tc.tile_pool(name="bound", bufs=1) as boundp,
            tc.tile_pool(name="ob", bufs=1) as obsp,
        ):
            # ---- input DMAs, one ordered queue (gpsimd): arrival order
            # must track the bulk loop's kp consumption order.
            xTt = [xtp.tile([P, KP, 2, P], f8, name=f"xt{b}")
                   for b in range(NB)]
            mt = [maskp.tile([P, 2, JL], dt.uint8, name=f"mk{kp}")
                  for kp in range(KP)]
            xrmt = [xrmp.tile([P, D], f8, name=f"xr{b}") for b in range(NB)]
            thb = constp.tile([P, JL], f32)

            nc.gpsimd.dma_start(xTt[0][:], xT_d[0])
            nc.gpsimd.dma_start(mt[0][:], m_d[0])
            nc.gpsimd.dma_start(xTt[1][:], xT_d[1])
            for kp in range(1, KP):
                nc.gpsimd.dma_start(mt[kp][:], m_d[kp])
            nc.gpsimd.dma_start(thb[:], thb_d[:])
            nc.gpsimd.dma_start(xTt[2][:], xT_d[2])
            nc.gpsimd.dma_start(xTt[3][:], xT_d[3])
            for b in range(NB):
                nc.gpsimd.dma_start(xrmt[b][:], xrm_d[b])

            # ---- constants / warm-up
            wtile = constp.tile([P, 2, P], f8)
            nc.vector.memset(wtile[:], 0.0)
            zero1 = constp.tile([P, 1], f32)
            nc.vector.memset(zero1[:], 0.0)
            neg4 = constp.tile([P, 1], f32)
            nc.vector.memset(neg4[:], -4.0)
            actw = constp.tile([P, 1], f32)
            nc.scalar.activation(actw[:], zero1[:], AF.Identity,
                                 bias=zero1[:], scale=1.0)

            rxa = [constp.tile([P, 1], f32, name=f"rxa{b}")
                   for b in range(NB)]
            rxe = [constp.tile([P, 1], f32, name=f"rxe{b}")
                   for b in range(NB)]
            sc8 = [actp.tile([P, D], f8, name=f"sc8_{i}") for i in range(3)]

            # rowsum reductions: b0/b1/b3 serial on scalar, b2 on vector
            # (emitted later, after the wave-0 psum-releasing ops).
            for b in (0, 1, 3):
                sct = sc8[0] if b == 3 else sc8[b]
                nc.scalar.activation(sct[:], xrmt[b][:], AF.Identity,
                                     bias=zero1[:], scale=1.0,
                                     accum_out=rxa[b][:])
                nc.scalar.activation(rxe[b][:], rxa[b][:], AF.Identity,
                                     bias=neg4[:], scale=1.0 / 1024.0)

            obs = [obsp.tile([P, JL], dt.uint8, name=f"ob{b}")
                   for b in range(NB)]

            with tc.tile_pool(name="psacc", bufs=1, space="PSUM") as psacc:
                dps = psacc.tile([P, JN], f32, tag="acc0", name="dps")
                for i in range(WARM):
                    nc.tensor.matmul(dps[:, 0:P], wtile[:], wtile[:],
                                     start=True, stop=True, perf_mode=DR)

                for w in range(2):
                    bs = (2 * w, 2 * w + 1)
                    ps = {}
                    for b2 in range(2):
                        for j4 in range(JT):
                            ps[(b2, j4)] = psacc.tile(
                                [P, JN], f32, tag=f"acc{b2 * JT + j4}",
                                name=f"acc_w{w}_{b2}_{j4}")
                    # bulk: kp-major over kp 0..11
                    for kp in range(KP - KRET):
                        for b2 in range(2):
                            wap = xTt[bs[b2]][:, kp, :, :]
                            for j4 in range(JT):
                                nc.tensor.matmul(
                                    ps[(b2, j4)][:], wap,
                                    mt[kp][:, :,
                                           j4 * JN:(j4 + 1) * JN].bitcast(f8),
                                    start=(kp == 0), stop=False,
                                    perf_mode=DR)
                    # retirement: group-major over kp 12..15, staggered
                    tmps = []
                    for b2 in range(2):
                        b = bs[b2]
                        for j4 in range(JT):
                            jj = j4 * JN
                            for kp in range(KP - KRET, KP):
                                nc.tensor.matmul(
                                    ps[(b2, j4)][:], xTt[b][:, kp, :, :],
                                    mt[kp][:, :, jj:jj + JN].bitcast(f8),
                                    start=False, stop=(kp == KP - 1),
                                    perf_mode=DR)
                            if w == 0:
                                # two-op epilogue: op1 releases the psum
                                # bank using only thb; op2 (emitted below)
                                # waits on the late rowsum path.
                                tmp = tmpp.tile([P, JN], f32,
                                                tag=f"tmp{b2 * JT + j4}",
                                                name=f"tmp{b}_{j4}")
                                nc.vector.tensor_tensor(
                                    tmp[:], ps[(b2, j4)][:],
                                    thb[:, jj:jj + JN], op=ALU.subtract)
                                tmps.append((b, jj, tmp))
                            else:
                                nc.vector.tensor_tensor(
                                    obs[b][:, jj:jj + JN], ps[(b2, j4)][:],
                                    bound[(b, j4)][:], op=ALU.is_gt)
                                nc.sync.dma_start(o_d[b, :, jj:jj + JN],
                                                  obs[b][:, jj:jj + JN])
                    if w == 0:
                        # b2 rowsum, wave-0 op2s, then wave-1 bound tiles
                        # -- all on DVE, emitted after wave-0's op1s so
                        # they can't head-of-line block the psum-bank
                        # releases.
                        nc.vector.tensor_scalar(
                            sc8[2][:], xrmt[2][:], 1.0, 0.0,
                            op0=ALU.mult, op1=ALU.add,
                            accum_out=rxa[2][:])
                        nc.vector.tensor_scalar(
                            rxe[2][:], rxa[2][:], 1.0 / 1024.0, -4.0,
                            op0=ALU.mult, op1=ALU.add)
                        for b, jj, tmp in tmps:
                            nc.vector.tensor_scalar(
                                obs[b][:, jj:jj + JN], tmp[:],
                                rxe[b][:], None, op0=ALU.is_gt)
                            nc.sync.dma_start(o_d[b, :, jj:jj + JN],
                                              obs[b][:, jj:jj + JN])
                        bound = {}
                        for b in (2, 3):
                            for j4 in range(JT):
                                bt = boundp.tile([P, JN], f32,
                                                 name=f"bnd{b}_{j4}")
                                nc.vector.tensor_scalar(
                                    bt[:], thb[:, j4 * JN:(j4 + 1) * JN],
                                    rxe[b][:], None, op0=ALU.add)
                                bound[(b, j4)] = bt

    nc.compile()
    return nc


def _get_nc():
    if "nc" not in _cache:
        _cache["nc"] = _build()
    return _cache["nc"]


def _prep_core(xs8, mask_buf, thb_buf):
    """Per-core input dict from the fp8 x slice and shared mask/th bufs."""
    t = xs8.reshape(NB, P, KP, 2, P)            # [b, m, kp, ko, ki]
    xT_buf = np.ascontiguousarray(t.transpose(0, 4, 2, 3, 1))
    return {
        "xT": xT_buf,                            # [b, ki, kp, ko, m]
        "xrm": np.ascontiguousarray(xs8.reshape(NB, P, D)),
        "masks": mask_buf,
        "thb": thb_buf,
    }


def run(x, masks, thresholds, trace=False):
    """Run the SPMD kernel on 8 cores. Returns (out_bool, results)."""
    import ml_dtypes
    from concourse.bass_utils import run_bass_kernel_spmd

    nc = _get_nc()
    f8 = ml_dtypes.float8_e4m3

    xs8_all = np.where(np.asarray(x) != 0, np.float32(1.0),
                       np.float32(-1.0)).astype(f8)
    m_u8 = np.ascontiguousarray(np.asarray(masks).view(np.uint8))
    th = np.asarray(thresholds).astype(np.float32) * np.float32(2.0 ** -9)

    mask_bufs, thb_bufs = [], []
    for h in range(GJ):
        mh = m_u8[:, h * JL:(h + 1) * JL].reshape(KP, 2, P, JL)
        mask_bufs.append(np.ascontiguousarray(mh.transpose(0, 2, 1, 3)))
        thb_bufs.append(np.ascontiguousarray(
            np.broadcast_to(th[None, h * JL:(h + 1) * JL], (P, JL))))

    in_maps = []
    for c in range(NCORES):
        g, h = c // GJ, c % GJ
        in_maps.append(_prep_core(xs8_all[g * ML:(g + 1) * ML],
                                  mask_bufs[h], thb_bufs[h]))

    res = run_bass_kernel_spmd(nc, in_maps, core_ids=list(range(NCORES)),
                               trace=trace)
    out = np.empty((B, J), dtype=np.uint8)
    for c in range(NCORES):
        g, h = c // GJ, c % GJ
        out[g * ML:(g + 1) * ML, h * JL:(h + 1) * JL] = \
            res.results[c]["out"].reshape(ML, JL)
    return out.view(np.bool_), res


def kernel(x, masks, thresholds):
    x = np.asarray(x)
    masks = np.asarray(masks)
    thresholds = np.asarray(thresholds)
    out, _ = run(x, masks, thresholds, trace=False)
    return out
